# revision 3
# baseline (speedup 1.0000x reference)
"""Baichuan-13B attention block (QKV packed proj + ALiBi causal attention via
identity paged-KV roundtrip + o_proj), tensor-parallel over 8 TRN2 NeuronCores.

Sharding: heads are split 5-per-core (w_pack column shards per interleaved
q/k/v head groups, o_proj row shards); attention outputs are AllGathered in a
feature-major (D-major / transposed) layout, and each core computes a disjoint
640-column slice of the final output, concatenated on the host.

The paged-KV cache fill + gather in the reference is an identity mapping:
the caches start zeroed, the block table (fill=arange) is injective, and the
gather reads back exactly the freshly written K/V. So attention consumes the
projected K/V directly.

All matmuls run in bf16 (fp32 PSUM accumulation). Softmax uses the exact
max-free rewrite exp(s - slope*q) with per-q shift folded in f32; masked
terms are zeroed multiplicatively post-exp.
"""

import math

import numpy as np
import ml_dtypes

import concourse.bass as bass
import concourse.mybir as mybir
import concourse.tile as tile
from concourse import bacc
from concourse.bass_utils import run_bass_kernel_spmd

# ---- problem constants (hardcoded per contract) ----
B, S = 2, 2048
HID, H, D = 5120, 40, 128
N_CORES = 8
HL = H // N_CORES            # 5 local heads
FL = HL * D                  # 640 local features
T = B * S                    # 4096 tokens
SCALE = 1.0 / math.sqrt(D)

BF16 = mybir.dt.bfloat16
F32 = mybir.dt.float32
NPBF16 = ml_dtypes.bfloat16

LAST_EXEC_NS = None


def _alibi_slopes(n):
    def pow2_slopes(m):
        start = 2.0 ** (-(2.0 ** -(math.log2(m) - 3)))
        return [start * (start ** i) for i in range(m)]
    if math.log2(n).is_integer():
        return pow2_slopes(int(n))
    m = 2 ** math.floor(math.log2(n))
    return pow2_slopes(m) + pow2_slopes(2 * m)[0::2][: n - m]


def _build_nc():
    nc = bacc.Bacc(num_devices=N_CORES)

    hT = nc.declare_dram_parameter("hT", [HID, T], BF16, isOutput=False)
    wqkT = nc.declare_dram_parameter("wqkT", [HID, 2 * FL], BF16, isOutput=False)
    wvT = nc.declare_dram_parameter("wvT", [HID, FL], BF16, isOutput=False)
    owT = nc.declare_dram_parameter("owT", [HID, FL], BF16, isOutput=False)
    rowvec = nc.declare_dram_parameter("rowvec", [HL, 128, S], F32, isOutput=False)
    biascol = nc.declare_dram_parameter("biascol", [HL, 128, S // 128], F32, isOutput=False)
    masks = nc.declare_dram_parameter("masks", [4, 128, 512], F32, isOutput=False)
    ones_k = nc.declare_dram_parameter("ones_k", [128, 1], BF16, isOutput=False)
    ones_1 = nc.declare_dram_parameter("ones_1", [1, 128], F32, isOutput=False)
    out = nc.declare_dram_parameter("out", [T, FL], F32, isOutput=True)

    # internal DRAM scratch
    qkT = nc.dram_tensor("qkT", [2 * FL, T], BF16)          # rows: [q feats | k feats]
    vtok = nc.dram_tensor("vtok", [HL, T, D], BF16)          # token-major V per head
    attnT_local = nc.dram_tensor("attnT_local", [FL, T], BF16)
    attnT_full = nc.dram_tensor("attnT_full", [H * D, T], BF16, addr_space="Shared")

    CT = HID // 128  # 40 contraction chunks
    NTT = T // 512   # 8 token tiles of 512

    with tile.TileContext(nc) as tc:
        # ---------- Phase A: QKV projection ----------
        # q and k passes: D-major output qkT[f, t] with w tiles stationary.
        for pidx in range(2):  # 0 = q, 1 = k
            with (
                tc.tile_pool(name=f"wA{pidx}", bufs=1) as wpool,
                tc.tile_pool(name=f"sA{pidx}", bufs=2) as spool,
                tc.tile_pool(name=f"pA{pidx}", bufs=4, space="PSUM") as ppool,
                tc.tile_pool(name=f"eA{pidx}", bufs=4) as epool,
            ):
                wt = wpool.tile([128, CT, FL], BF16, name=f"wt{pidx}")
                nc.sync.dma_start(
                    wt[:],
                    wqkT[:, pidx * FL:(pidx + 1) * FL].rearrange("(o p) f -> p o f", p=128),
                )
                for tt in range(NTT):
                    slab = spool.tile([128, CT, 512], BF16, tag="slab", name=f"slab{pidx}_{tt}")
                    nc.sync.dma_start(
                        slab[:],
                        hT[:, 512 * tt:512 * (tt + 1)].rearrange("(o p) t -> p o t", p=128),
                    )
                    for ft in range(HL):
                        ps = ppool.tile([128, 512], F32, tag="ps", name=f"psA{pidx}_{tt}_{ft}")
                        for ct in range(CT):
                            nc.tensor.matmul(
                                ps[:],
                                wt[:, ct, 128 * ft:128 * (ft + 1)],
                                slab[:, ct, :],
                                start=(ct == 0),
                                stop=(ct == CT - 1),
                            )
                        ev = epool.tile([128, 512], BF16, tag="ev", name=f"evA{pidx}_{tt}_{ft}")
                        nc.scalar.copy(ev[:], ps[:])
                        nc.sync.dma_start(
                            qkT[pidx * FL + 128 * ft: pidx * FL + 128 * (ft + 1),
                                512 * tt:512 * (tt + 1)],
                            ev[:],
                        )

        # v pass: token-major output with hidden tiles stationary.
        with (
            tc.tile_pool(name="wV", bufs=1) as wpool,
            tc.tile_pool(name="sV", bufs=2) as spool,
            tc.tile_pool(name="pV", bufs=2, space="PSUM") as ppool,
            tc.tile_pool(name="eV", bufs=3) as epool,
        ):
            wv = wpool.tile([128, CT, FL], BF16, name="wv")
            nc.sync.dma_start(wv[:], wvT[:].rearrange("(o p) f -> p o f", p=128))
            for tt in range(NTT):
                slabv = spool.tile([128, CT, 512], BF16, tag="slabv", name=f"slabv{tt}")
                nc.sync.dma_start(
                    slabv[:],
                    hT[:, 512 * tt:512 * (tt + 1)].rearrange("(o p) t -> p o t", p=128),
                )
                for tc4 in range(4):
                    psv = ppool.tile([128, FL], F32, tag="psv", name=f"psv{tt}_{tc4}")
                    for ct in range(CT):
                        nc.tensor.matmul(
                            psv[:, 0:512],
                            slabv[:, ct, 128 * tc4:128 * (tc4 + 1)],
                            wv[:, ct, 0:512],
                            start=(ct == 0), stop=(ct == CT - 1),
                        )
                        nc.tensor.matmul(
                            psv[:, 512:FL],
                            slabv[:, ct, 128 * tc4:128 * (tc4 + 1)],
                            wv[:, ct, 512:FL],
                            start=(ct == 0), stop=(ct == CT - 1),
                        )
                    evv = epool.tile([128, FL], BF16, tag="evv", name=f"evv{tt}_{tc4}")
                    nc.scalar.copy(evv[:], psv[:])
                    tglob = 4 * tt + tc4
                    for hl in range(HL):
                        nc.sync.dma_start(
                            vtok[hl, 128 * tglob:128 * (tglob + 1), :],
                            evv[:, 128 * hl:128 * (hl + 1)],
                        )

        # ---------- Phase B: attention per (head, batch) ----------
        with (
            tc.tile_pool(name="constB", bufs=1) as cpool,
            tc.tile_pool(name="ioB", bufs=2) as iopool,
            tc.tile_pool(name="workB", bufs=4) as wkpool,
            tc.tile_pool(name="psS", bufs=2, space="PSUM") as psS,
            tc.tile_pool(name="psO", bufs=2, space="PSUM") as psO,
            tc.tile_pool(name="psR", bufs=2, space="PSUM") as psR,
            tc.tile_pool(name="psB", bufs=2, space="PSUM") as psB,
        ):
            masks_sb = cpool.tile([128, 4, 512], F32, name="masks_sb")
            nc.sync.dma_start(masks_sb[:], masks[:].rearrange("m p q -> p m q"))
            onesk_sb = cpool.tile([128, 1], BF16, name="onesk_sb")
            nc.sync.dma_start(onesk_sb[:], ones_k[:])
            ones1_sb = cpool.tile([1, 128], F32, name="ones1_sb")
            nc.sync.dma_start(ones1_sb[:], ones_1[:])

            NKC = S // 128  # 16 k-chunks per sequence
            for hl in range(HL):
                for b in range(B):
                    kTt = iopool.tile([128, S], BF16, tag="kTt", name=f"kTt{hl}_{b}")
                    nc.sync.dma_start(
                        kTt[:], qkT[FL + 128 * hl: FL + 128 * (hl + 1), S * b:S * (b + 1)]
                    )
                    qTt = iopool.tile([128, S], BF16, tag="qTt", name=f"qTt{hl}_{b}")
                    nc.sync.dma_start(
                        qTt[:], qkT[128 * hl:128 * (hl + 1), S * b:S * (b + 1)]
                    )
                    vt = iopool.tile([128, NKC, D], BF16, tag="vt", name=f"vt{hl}_{b}")
                    nc.sync.dma_start(
                        vt[:], vtok[hl, S * b:S * (b + 1), :].rearrange("(o p) d -> p o d", p=128)
                    )
                    rv = iopool.tile([128, S], F32, tag="rv", name=f"rv{hl}_{b}")
                    nc.sync.dma_start(rv[:], rowvec[hl])
                    bc = iopool.tile([128, NKC], F32, tag="bc", name=f"bc{hl}_{b}")
                    nc.sync.dma_start(bc[:], biascol[hl])

                    for j in range(S // 512):  # q-tiles of 512
                        nkc = 4 * (j + 1)     # causal: k-chunks 0..4j+3
                        po = psO.tile([128, 512], F32, tag="po", name=f"po{hl}_{b}_{j}")
                        pr = psR.tile([1, 512], F32, tag="pr", name=f"pr{hl}_{b}_{j}")
                        for i in range(nkc):
                            ps = psS.tile([128, 512], F32, tag="ps", name=f"psB{hl}_{b}_{j}_{i}")
                            nc.tensor.matmul(
                                ps[:],
                                kTt[:, 128 * i:128 * (i + 1)],
                                qTt[:, 512 * j:512 * (j + 1)],
                                start=True, stop=True,
                            )
                            tmp = wkpool.tile([128, 512], F32, tag="tmp", name=f"tmp{hl}_{b}_{j}_{i}")
                            nc.vector.tensor_add(tmp[:], ps[:], rv[:, 512 * j:512 * (j + 1)])
                            if i >= 4 * j:  # diagonal block: additive causal mask (-1e9)
                                nc.vector.tensor_add(tmp[:], tmp[:], masks_sb[:, i - 4 * j, :])
                            pt = wkpool.tile([128, 512], BF16, tag="pt", name=f"pt{hl}_{b}_{j}_{i}")
                            nc.scalar.activation(
                                pt[:], tmp[:], mybir.ActivationFunctionType.Exp,
                                bias=bc[:, i:i + 1], scale=1.0,
                            )
                            nc.tensor.matmul(
                                po[:], vt[:, i, :], pt[:],
                                start=(i == 0), stop=(i == nkc - 1),
                            )
                            nc.tensor.matmul(
                                pr[:], onesk_sb[:], pt[:],
                                start=(i == 0), stop=(i == nkc - 1),
                            )
                        recip = wkpool.tile([1, 512], F32, tag="recip", name=f"recip{hl}_{b}_{j}")
                        nc.vector.reciprocal(recip[:], pr[:])
                        pb = psB.tile([128, 512], F32, tag="pb", name=f"pb{hl}_{b}_{j}")
                        nc.tensor.matmul(pb[:], ones1_sb[:], recip[:], start=True, stop=True)
                        pbs = wkpool.tile([128, 512], F32, tag="pbs", name=f"pbs{hl}_{b}_{j}")
                        nc.scalar.copy(pbs[:], pb[:])
                        ao = wkpool.tile([128, 512], BF16, tag="ao", name=f"ao{hl}_{b}_{j}")
                        nc.vector.tensor_mul(ao[:], po[:], pbs[:])
                        nc.sync.dma_start(
                            attnT_local[128 * hl:128 * (hl + 1),
                                        S * b + 512 * j: S * b + 512 * (j + 1)],
                            ao[:],
                        )

    # ---------- AllGather (between tile contexts, raw semaphores) ----------
    with nc.semaphore("cc_sem") as cc_sem:
        nc.gpsimd.collective_compute(
            "AllGather",
            mybir.AluOpType.bypass,
            ins=[attnT_local[:]],
            outs=[attnT_full[:]],
            replica_groups=[list(range(N_CORES))],
        ).then_inc(cc_sem, 1)
        nc.gpsimd.wait_ge(cc_sem, 1)
        nc.all_engine_barrier()

    # ---------- Phase C: o_proj slice ----------
    FCT = H * D // 128  # 40 feature chunks
    with tile.TileContext(nc) as tc2:
        with (
            tc2.tile_pool(name="wC", bufs=1) as wpool,
            tc2.tile_pool(name="sC", bufs=2) as spool,
            tc2.tile_pool(name="pC", bufs=2, space="PSUM") as ppool,
            tc2.tile_pool(name="eC", bufs=3) as epool,
        ):
            ow = wpool.tile([128, FCT, FL], BF16, name="ow")
            nc.sync.dma_start(ow[:], owT[:].rearrange("(o p) f -> p o f", p=128))
            for tt in range(NTT):
                slab = spool.tile([128, FCT, 512], BF16, tag="slabC", name=f"slabC{tt}")
                nc.sync.dma_start(
                    slab[:],
                    attnT_full[:, 512 * tt:512 * (tt + 1)].rearrange("(o p) t -> p o t", p=128),
                )
                for tc4 in range(4):
                    psc = ppool.tile([128, FL], F32, tag="psc", name=f"psc{tt}_{tc4}")
                    for fc in range(FCT):
                        nc.tensor.matmul(
                            psc[:, 0:512],
                            slab[:, fc, 128 * tc4:128 * (tc4 + 1)],
                            ow[:, fc, 0:512],
                            start=(fc == 0), stop=(fc == FCT - 1),
                        )
                        nc.tensor.matmul(
                            psc[:, 512:FL],
                            slab[:, fc, 128 * tc4:128 * (tc4 + 1)],
                            ow[:, fc, 512:FL],
                            start=(fc == 0), stop=(fc == FCT - 1),
                        )
                    ev = epool.tile([128, FL], F32, tag="evC", name=f"evC{tt}_{tc4}")
                    nc.scalar.copy(ev[:], psc[:])
                    tglob = 4 * tt + tc4
                    nc.sync.dma_start(out[128 * tglob:128 * (tglob + 1), :], ev[:])

    return nc


_NC = None


def _get_nc():
    global _NC
    if _NC is None:
        nc = _build_nc()
        nc.finalize()
        _NC = nc
    return _NC


def _prep_in_maps(hidden_states, w_pack, o_proj_w):
    slopes = np.asarray(_alibi_slopes(H), dtype=np.float64)
    hT = np.ascontiguousarray(hidden_states.T).astype(NPBF16)

    # shared constants
    kk = np.arange(128)
    qq = np.arange(512)
    masks = np.zeros((4, 128, 512), dtype=np.float32)
    for m in range(4):
        masks[m] = np.where((128 * m + kk)[:, None] <= qq[None, :], 0.0, -1e9
                            ).astype(np.float32)
    ones_k = np.ones((128, 1), dtype=NPBF16)
    ones_1 = np.ones((1, 128), dtype=np.float32)

    in_maps = []
    for c in range(N_CORES):
        fsl = slice(FL * c, FL * (c + 1))
        q_rows = w_pack[fsl].astype(np.float32) * SCALE
        k_rows = w_pack[HID + FL * c: HID + FL * (c + 1)]
        v_rows = w_pack[2 * HID + FL * c: 2 * HID + FL * (c + 1)]
        wqkT = np.ascontiguousarray(
            np.concatenate([q_rows, k_rows], axis=0).T
        ).astype(NPBF16)
        wvT = np.ascontiguousarray(v_rows.T).astype(NPBF16)
        owT = np.ascontiguousarray(o_proj_w[fsl].T).astype(NPBF16)

        sl = slopes[HL * c: HL * (c + 1)]
        qpos = np.arange(S, dtype=np.float64)
        rowvec = np.broadcast_to(
            (-sl[:, None] * qpos[None, :])[:, None, :], (HL, 128, S)
        ).astype(np.float32)
        rowvec = np.ascontiguousarray(rowvec)
        ii = np.arange(S // 128, dtype=np.float64)
        biascol = (sl[:, None, None] * (128.0 * ii[None, None, :] + kk[None, :, None])
                   ).astype(np.float32)

        in_maps.append({
            "hT": hT,
            "wqkT": wqkT,
            "wvT": wvT,
            "owT": owT,
            "rowvec": rowvec,
            "biascol": np.ascontiguousarray(biascol),
            "masks": masks,
            "ones_k": ones_k,
            "ones_1": ones_1,
        })
    return in_maps


def _run(hidden_states, w_pack, o_proj_w, trace=False):
    global LAST_EXEC_NS
    nc = _get_nc()
    in_maps = _prep_in_maps(hidden_states, w_pack, o_proj_w)
    res = run_bass_kernel_spmd(
        nc, in_maps, core_ids=list(range(N_CORES)), trace=trace
    )
    LAST_EXEC_NS = res.exec_time_ns
    out = np.concatenate([res.results[c]["out"] for c in range(N_CORES)], axis=1)
    return np.ascontiguousarray(out.astype(np.float32))


def kernel(hidden_states, w_pack, o_proj_w, k_cache, v_cache, block_offsets,
           **_ignored):
    # The paged cache roundtrip (zero-filled caches + injective arange block
    # table, written then gathered with the same offsets) is an identity, so
    # k_cache / v_cache / block_offsets do not affect the output.
    hidden_states = np.asarray(hidden_states, dtype=np.float32)
    w_pack = np.asarray(w_pack, dtype=np.float32)
    o_proj_w = np.asarray(o_proj_w, dtype=np.float32)
    return _run(hidden_states, w_pack, o_proj_w, trace=False)


def kernel_traced(hidden_states, w_pack, o_proj_w, k_cache=None, v_cache=None,
                  block_offsets=None, **_ignored):
    hidden_states = np.asarray(hidden_states, dtype=np.float32)
    w_pack = np.asarray(w_pack, dtype=np.float32)
    o_proj_w = np.asarray(o_proj_w, dtype=np.float32)
    return _run(hidden_states, w_pack, o_proj_w, trace=True)
